# revision 44
# baseline (speedup 1.0000x reference)
"""Self-attention (Base_OC / SAGAN-style) module on Trainium2, 8 NeuronCores.

Problem: x[4, 64, 64, 512]; per batch element b (N = 4096 tokens, C = 512):
  f = x@wf+bf [N,64]; g = x@wg+bg [N,64]; hv = x@wh+bh [N,256]
  s = g @ f^T [N,N]; beta = softmax(s); o = beta @ hv [N,256]
  att = gamma*(o@wo+bo) + x; y = relu(BN([att,x] @ wc + bc))

Sharding: 8 cores = batch(4) x query-row-halves(2). Each core receives x[b]
permuted so its own 2048 query rows come first (attention is permutation-
invariant over keys), computes the pipeline for those rows, returns yT
[512, 2048] (un-transposed on the host).

Design notes (evolved from an fp32r v1 at 256us to ~177us; tensor engine is
the bottleneck at ~64% MFU with all other engines well under it):
 - All matmul operands are bf16: same 1 col/cycle PE rate as fp32r>=256-wide
   but ~4x less PE multiplier power, which stops the mid-kernel HAM 4/8
   clock-gate windows the fp32r version suffered, and enables FWL fast
   weight loads (disabled for fp32 modes).
 - exp() splits across engines per s-pair: half on DVE as a single
   tensor_scalar (Schraudolph: bf16 bits = int16 round of a*x+b, ~3% rel
   err), half exact on ACT — neither engine alone keeps up with the PE's
   u-matmul rate. Softmax skips max-subtraction (|logit| <~ 70, exp stays in
   bf16/fp32 range); the denominator comes from a ones-column in hv.
 - Bias/residual algebra folded into host-prepped weights: bh vanishes
   (sum(beta)=1 => beta@(hv+bh) = beta@hv + bh, folded into the final bias
   row), gamma folds into wo, and the +x residual folds into the wc x-half
   (y = z'@wc1 + x@(wc1+wc2)), so z' needs only a PSUM->bf16 cast.
 - o-transpose (q,d2)->(d2,q) runs on the DMA xbar (dma_start_transpose,
   3D-out form: both et chunks in one descriptor), not the PE. All DMAs are
   batched into strided single descriptors (each DMA costs ~600ns of shared
   ring time regardless of size) and the y-output DMAs stay on the sync
   queue behind the transposes (a DMA on the ACT queue head-of-line-blocks
   the relus while waiting for the shared DMA path).
 - s/u pipelining: two K=64 s-matmuls run concurrently in the PE via
   tile_position row packing; B1 issues two s-pairs then two u-blocks per
   cycle (s<->u transitions on the array cost ~200ns each since an s-pair
   occupies both row halves; exp_pool bufs=8 keeps 4 pairs in flight).
 - y is computed as yT[c,q] (stationary = wc chunk, moving = z'/xT) so the
   BN-folded bias + relu is one per-partition ACT op; host un-transposes.
 - Phases: A streams x in (2-block prefetch) and computes f/g/hv plus query
   block 0's s/exp/u; B1 runs s/exp/u for blocks 1-3; B2 does z=o@wo and y
   (PSUM-bank-interleaved matmul emission; oT transposes prefetched during
   B1).
"""

import numpy as np
import ml_dtypes

import concourse.bacc as bacc
import concourse.mybir as mybir
import concourse.tile as tile
from concourse.bass_utils import run_bass_kernel_spmd

FP = mybir.dt.float32
BF = mybir.dt.bfloat16
I16 = mybir.dt.int16
AF = mybir.ActivationFunctionType
OP = mybir.AluOpType

N_FULL, N_OWN, C, D8, D2 = 4096, 2048, 512, 64, 256
NMT = N_FULL // 128   # 32 key tiles
NCT = C // 128        # 4 channel tiles
NET = D2 // 128       # 2 e tiles
NNB = N_OWN // 512    # 4 query blocks per core
HW2 = 258             # hv width: 256 values | ones col | pad
EPS = 1e-3

# Schraudolph exp in bf16 bits: bits16 = int(EA*x + EB) reinterpreted as bf16
# gives ~exp(x) with ~3% max relative error. EA = 2^7/ln2; EB = 127*2^7
# - 5.6 (mean-error-centering) + 0.5 (float->int truncation compensation).
EA = 184.66496280094688
EB = 16250.9


def build_program():
    nc = bacc.Bacc("TRN2", target_bir_lowering=False, debug=False, num_devices=8)

    xt_d = nc.dram_tensor("xt", [C, N_FULL], BF, kind="ExternalInput").ap()
    wfg_d = nc.dram_tensor("wfg", [C, 128], BF, kind="ExternalInput").ap()
    bfg_d = nc.dram_tensor("bfg", [128, 1], FP, kind="ExternalInput").ap()
    whx_d = nc.dram_tensor("whx", [C, HW2], BF, kind="ExternalInput").ap()
    wox_d = nc.dram_tensor("wox", [D2, C], BF, kind="ExternalInput").ap()
    wcx_d = nc.dram_tensor("wcx", [2 * C, C], BF, kind="ExternalInput").ap()
    bcb_d = nc.dram_tensor("bcb", [128, NCT], FP, kind="ExternalInput").ap()
    y_d = nc.dram_tensor("y", [C, N_OWN], FP, kind="ExternalOutput").ap()

    with tile.TileContext(nc) as tc:
        with (
            tc.tile_pool(name="consts", bufs=1) as cpool,
            tc.tile_pool(name="big", bufs=1) as bigp,
            tc.tile_pool(name="exps", bufs=8) as exp_pool,
            tc.tile_pool(name="sstream", bufs=2) as spool_sb,
            tc.tile_pool(name="psB_u", bufs=1, space="PSUM") as pu,
        ):
            xT = bigp.tile([128, NCT * N_FULL], BF)   # 32 KB/part
            fT = bigp.tile([128, N_FULL], BF)         # rows 0:64 f, 64:128 dup
            gT = bigp.tile([128, N_OWN], BF)          # rows 64:128 g, 0:64 dup
            hv = bigp.tile([128, NMT * HW2], BF)      # 16.5 KB/part
            ob = bigp.tile([128, NNB * 1024], BF)     # normalized o, [q, d2]
            oT = bigp.tile([128, NNB * 1024], BF)     # o transposed, [d2, q]
            whx_sb = cpool.tile([128, NCT * HW2], BF)
            wfg_sb = cpool.tile([128, NCT * 128], BF)
            bfg_sb = cpool.tile([128, 1], FP)

            def dma_xt(ch):
                # one strided descriptor covering all 4 channel tiles
                nc.sync.dma_start(
                    xT.rearrange("p (t n) -> p t n", t=NCT)
                    [:, :, ch * 512:(ch + 1) * 512],
                    xt_d.rearrange("(t p) n -> p t n", p=128)
                    [:, :, ch * 512:(ch + 1) * 512])

            # critical-path-first DMA order, one strided descriptor each:
            # only wfg + x block 0 gate the first fg matmuls (bfg/whx are
            # emitted after those matmuls so the counting semaphore the
            # matmuls wait on covers just these two descriptors).
            nc.sync.dma_start(wfg_sb.rearrange("p (t d) -> p t d", t=NCT),
                              wfg_d.rearrange("(t p) d -> p t d", p=128))
            dma_xt(0)

            # ones columns of hv (denominator trick), written once
            nc.vector.memset(
                hv.rearrange("p (t w) -> p t w", t=NMT)[:, :, D2:D2 + 1], 1.0)
            nc.vector.memset(
                hv.rearrange("p (t w) -> p t w", t=NMT)[:, :, D2 + 1:], 0.0)

            def emit_hv(mt, phv):
                hp = phv.tile([128, D2], FP, tag="hv")
                for ct in range(NCT):
                    nc.tensor.matmul(
                        hp,
                        xT[:, ct * N_FULL + mt * 128: ct * N_FULL + (mt + 1) * 128],
                        whx_sb[:, ct * HW2: ct * HW2 + D2],
                        start=(ct == 0), stop=(ct == NCT - 1))
                # plain copy PSUM->SBUF, alternating DVE/ACT (bias bh is
                # folded into bcb host-side since sum(beta) = 1)
                if mt % 2 == 0:
                    nc.vector.tensor_copy(hv[:, mt * HW2: mt * HW2 + D2], hp)
                else:
                    nc.scalar.copy(hv[:, mt * HW2: mt * HW2 + D2], hp)

            def emit_fg(ch, pfg, first=False):
                cs = slice(ch * 512, (ch + 1) * 512)
                if ch < NNB:
                    # packed [f|g]: out rows 0:64 = f, 64:128 = g
                    fgp = pfg.tile([128, 512], FP, tag="fg")
                    for ct in range(NCT):
                        nc.tensor.matmul(
                            fgp, wfg_sb[:, ct * 128:(ct + 1) * 128],
                            xT[:, ct * N_FULL + ch * 512:
                               ct * N_FULL + (ch + 1) * 512],
                            start=(ct == 0), stop=(ct == NCT - 1))
                    if first:
                        nc.sync.dma_start(bfg_sb, bfg_d)
                        nc.sync.dma_start(
                            whx_sb.rearrange("p (t d) -> p t d", t=NCT),
                            whx_d.rearrange("(t p) d -> p t d", p=128))
                    nc.scalar.activation(fT[0:D8, cs], fgp[0:D8, :],
                                         AF.Identity, bias=bfg_sb[0:D8, :])
                    nc.scalar.activation(gT[D8:128, cs], fgp[D8:128, :],
                                         AF.Identity, bias=bfg_sb[D8:128, :])
                    nc.sync.dma_start(fT[D8:128, cs], fT[0:D8, cs])
                    nc.sync.dma_start(gT[0:D8, cs], gT[D8:128, cs])
                else:
                    fp_ = pfg.tile([128, 512], FP, tag="fg")
                    for ct in range(NCT):
                        nc.tensor.matmul(
                            fp_[0:D8, :], wfg_sb[:, ct * 128: ct * 128 + D8],
                            xT[:, ct * N_FULL + ch * 512:
                               ct * N_FULL + (ch + 1) * 512],
                            start=(ct == 0), stop=(ct == NCT - 1))
                    nc.scalar.activation(fT[0:D8, cs], fp_[0:D8, :],
                                         AF.Identity, bias=bfg_sb[0:D8, :])
                    nc.sync.dma_start(fT[D8:128, cs], fT[0:D8, cs])

            def emit_s_pair(nb, mt2, spool, dve_both=False):
                # two K=64 s-matmuls concurrent in the PE array (row packing).
                # exp alternates engines per half: DVE Schraudolph (one
                # tensor_scalar, bf16 bits via int16) and exact ACT Exp —
                # neither engine alone can keep up with the PE's u-rate.
                nbs = slice(nb * 512, (nb + 1) * 512)
                exs = []
                for half in range(2):
                    mt = 2 * mt2 + half
                    lo, hi = (0, D8) if half == 0 else (D8, 128)
                    sps = spool.tile([128, 512], FP, tag="s")
                    nc.tensor.matmul(
                        sps, fT[lo:hi, mt * 128:(mt + 1) * 128],
                        gT[lo:hi, nbs], start=True, stop=True,
                        tile_position=(lo, 0))
                    if half == 0 or dve_both:
                        ex = exp_pool.tile([128, 512], I16, tag="expS")
                        nc.vector.tensor_scalar(ex, sps, EA, EB,
                                                op0=OP.mult, op1=OP.add)
                        exs.append(ex.bitcast(BF))
                    else:
                        ex = exp_pool.tile([128, 512], BF, tag="expS")
                        nc.scalar.activation(ex, sps, AF.Exp)
                        exs.append(ex)
                return exs

            def emit_u(mt2, exs, up):
                for half in range(2):
                    mt = 2 * mt2 + half
                    for ns in range(4):
                        nc.tensor.matmul(
                            up[:, ns * 512: ns * 512 + HW2],
                            exs[half][:, ns * 128:(ns + 1) * 128],
                            hv[:, mt * HW2:(mt + 1) * HW2],
                            start=(mt == 0), stop=(mt == NMT - 1))

            def emit_norm(nb, up, sp):
                # ob[q, d2] = up[q, :256] / denominator (col 256), bf16 out.
                # One strided reciprocal covers all 4 blocks; muls split
                # DVE/ACT so the up PSUM banks drain fast (next block's
                # u-matmuls reuse them).
                rcp4 = sp.tile([128, 4], FP, tag="rcp")
                nc.vector.reciprocal(
                    rcp4, up.rearrange("p (n w) -> p n w", n=4)[:, :, D2:D2 + 1])
                for ns in range(4):
                    obs = ob[:, nb * 1024 + ns * 256: nb * 1024 + (ns + 1) * 256]
                    ups = up[:, ns * 512: ns * 512 + 256]
                    if ns % 2 == 0:
                        nc.vector.tensor_scalar_mul(obs, ups, rcp4[:, ns:ns + 1])
                    else:
                        nc.scalar.mul(obs, ups, rcp4[:, ns:ns + 1])

            def emit_tp(nb):
                # one 3D-out xbar transpose per ns quarter: out[p, et, q] =
                # in[q, et*128+p] — both et chunks of oT in a single DMA.
                for ns in range(4):
                    nc.sync.dma_start_transpose(
                        oT[:, nb * 1024: (nb + 1) * 1024]
                        .rearrange("p (et q) -> p et q", et=NET)
                        [:, :, ns * 128:(ns + 1) * 128],
                        ob[:, nb * 1024 + ns * 256: nb * 1024 + (ns + 1) * 256])

            def emit_zy(nb, pz, py, sp, attp):
                # MM emission interleaves across output tiles (4 independent
                # PSUM banks) so consecutive PE ops never chain on one bank's
                # write port: same-bank accumulate chains measured 379 ns/MM
                # vs 216 warm.
                # z' = o @ (gamma*wo): gamma folded into wo host-side, and the
                # +x residual folded into the wc x-half (y = z'@wc1 +
                # x@(wc1+wc2)), so z' just needs a PSUM->bf16 cast.
                attT = attp.tile([128, NCT * 512], BF, tag="attT")
                for cth in range(2):
                    zps = [pz.tile([128, 512], FP, tag="z", name=f"zp{nb}_{cth}_{i}")
                           for i in range(2)]
                    for et in range(NET):
                        for i in range(2):
                            ct = 2 * cth + i
                            nc.tensor.matmul(
                                zps[i],
                                wox_sb[:, et * C + ct * 128: et * C + (ct + 1) * 128],
                                oT[:, nb * 1024 + et * 512: nb * 1024 + (et + 1) * 512],
                                start=(et == 0), stop=(et == NET - 1))
                    for i in range(2):
                        ct = 2 * cth + i
                        nc.vector.tensor_copy(
                            attT[:, ct * 512:(ct + 1) * 512], zps[i])
                ys = sp.tile([128, 4 * 512], FP, tag="ys")
                for coh in range(2):
                    yps = [py.tile([128, 512], FP, tag="y", name=f"yp{nb}_{coh}_{i}")
                           for i in range(2)]
                    # x-chunks first (independent of attT), att-chunks after
                    for k in (4, 5, 6, 7, 0, 1, 2, 3):
                        mov = (xT[:, (k - 4) * N_FULL + nb * 512:
                                  (k - 4) * N_FULL + (nb + 1) * 512] if k >= 4
                               else attT[:, k * 512:(k + 1) * 512])
                        for i in range(2):
                            co = 2 * coh + i
                            nc.tensor.matmul(
                                yps[i],
                                wcx_sb[:, k * C + co * 128: k * C + (co + 1) * 128],
                                mov, start=(k == 4), stop=(k == 3))
                    for i in range(2):
                        co = 2 * coh + i
                        nc.scalar.activation(ys[:, co * 512:(co + 1) * 512],
                                             yps[i], AF.Relu,
                                             bias=bcb_sb[:, co:co + 1])
                        # y DMA per co right after its relu, on sync (a DMA
                        # on the ACT queue would wait for pending transposes
                        # in the shared DMA path and block the relus).
                        nc.sync.dma_start(
                            y_d[co * 128:(co + 1) * 128,
                                nb * 512:(nb + 1) * 512],
                            ys[:, co * 512:(co + 1) * 512])

            # ---- phase A: projections + query block 0's s/exp/u pipeline ----
            with (
                tc.tile_pool(name="psA_fg", bufs=1, space="PSUM") as pfg,
                tc.tile_pool(name="psA_hv", bufs=1, space="PSUM") as phv,
                tc.tile_pool(name="psA_s", bufs=2, space="PSUM") as spA,
                nc.named_scope("phaseA"),
            ):
                up0 = pu.tile([128, 2048], FP, tag="u")
                for ch in range(8):
                    emit_fg(ch, pfg, first=(ch == 0))
                    if ch == 0:
                        dma_xt(1)
                    if ch < 6:
                        dma_xt(ch + 2)
                    emit_hv(4 * ch, phv)
                    exs_a = emit_s_pair(0, 2 * ch, spA)
                    emit_hv(4 * ch + 1, phv)
                    emit_u(2 * ch, exs_a, up0)
                    emit_hv(4 * ch + 2, phv)
                    exs_b = emit_s_pair(0, 2 * ch + 1, spA)
                    emit_hv(4 * ch + 3, phv)
                    if ch == 1:
                        wox_sb = cpool.tile([128, NET * C], BF)
                        nc.sync.dma_start(
                            wox_sb.rearrange("p (t d) -> p t d", t=NET),
                            wox_d.rearrange("(t p) d -> p t d", p=128))
                    if ch == 3:
                        wcx_sb = cpool.tile([128, 8 * C], BF)
                        nc.sync.dma_start(
                            wcx_sb.rearrange("p (t d) -> p t d", t=8),
                            wcx_d.rearrange("(t p) d -> p t d", p=128))
                        bcb_sb = cpool.tile([128, NCT], FP)
                        nc.sync.dma_start(bcb_sb, bcb_d)
                    emit_u(2 * ch + 1, exs_b, up0)
                emit_norm(0, up0, spool_sb)
                emit_tp(0)

            # ---- B1: query blocks 1-3, s-pair(i+1) issued before u(i); the
            # lookahead runs across nb boundaries so the up-PSUM drain
            # (norm) hides under the next block's first s-pair + u-block.
            with tc.tile_pool(name="psB_s", bufs=4, space="PSUM") as spB, \
                 nc.named_scope("phaseB1"):
                ups = {}
                prev = None

                def flush_prev(prev):
                    pnb, pmt2, pexs = prev
                    if pmt2 == 0:
                        ups[pnb] = pu.tile([128, 2048], FP, tag="u",
                                           name=f"up{pnb}")
                    emit_u(pmt2, pexs, ups[pnb])
                    if pmt2 == NMT // 2 - 1:
                        emit_norm(pnb, ups[pnb], spool_sb)
                        emit_tp(pnb)

                # two s-pairs then two u-blocks per cycle: halves the number
                # of s<->u transitions on the PE array (each costs ~200ns:
                # the next u-LDWEIGHTS can't overlap an s-pair occupying both
                # row halves). Needs exp_pool bufs=8 (4 pairs in flight).
                pairs = [(nb, mt2) for nb in range(1, NNB)
                         for mt2 in range(NMT // 2)]
                pending = []
                for j in range(0, len(pairs), 2):
                    for nb, mt2 in pairs[j:j + 2]:
                        # at block boundaries both exps ride the DVE so the
                        # ACT-side normalize muls can't delay them
                        exs = emit_s_pair(nb, mt2, spB, dve_both=(mt2 == 0))
                        pending.append((nb, mt2, exs))
                    while len(pending) > 2:
                        flush_prev(pending.pop(0))
                for prev in pending:
                    flush_prev(prev)

            # ---- B2: z = o@wo, att residual, y = relu([att,x]@wc + b) ----
            with (
                tc.tile_pool(name="psB_z", bufs=2, space="PSUM") as pz,
                tc.tile_pool(name="psB_y", bufs=2, space="PSUM") as py,
                tc.tile_pool(name="ystream", bufs=6) as ysp,
                tc.tile_pool(name="attp", bufs=2) as attp,
                nc.named_scope("phaseB2"),
            ):
                for nb in range(NNB):
                    emit_zy(nb, pz, py, ysp, attp)

    nc.compile()
    return nc


_PROG = None


def _get_prog():
    global _PROG
    if _PROG is None:
        _PROG = build_program()
    return _PROG


def make_in_maps(x, wf, bf, wg, bg, wh, bh, wo, bo, gamma, wc, bc,
                 bn_scale, bn_bias, bn_mean, bn_var):
    bf16 = lambda a: np.ascontiguousarray(np.asarray(a, np.float32)).astype(
        ml_dtypes.bfloat16)
    f32 = lambda a: np.ascontiguousarray(np.asarray(a, dtype=np.float32))
    x = np.asarray(x, np.float32)
    B = x.shape[0]
    xf = x.reshape(B, N_FULL, C)
    gv = float(np.asarray(gamma).ravel()[0])
    d = lambda a: np.asarray(a, np.float64)
    sp_ = d(bn_scale) / np.sqrt(d(bn_var) + EPS)
    wcx = d(wc) * sp_[None, :]
    # bias row: BN-folded bc, plus gamma*(bh@wo + bo) routed through the att
    # half of wc (sum(beta)=1 makes the hv bias a constant shift of o).
    bcrow = (d(bc) - d(bn_mean)) * sp_ + d(bn_bias)
    bcrow = bcrow + gv * (d(bh) @ d(wo) + d(bo)) @ wcx[:C]
    whx = np.concatenate([np.asarray(wh, np.float32),
                          np.zeros((C, 2), np.float32)], axis=1)
    common = dict(
        wfg=bf16(np.concatenate([np.asarray(wf, np.float32),
                                 np.asarray(wg, np.float32)], axis=1)),
        bfg=f32(np.concatenate([np.asarray(bf, np.float32).ravel(),
                                np.asarray(bg, np.float32).ravel()])).reshape(128, 1),
        whx=bf16(whx),
        wox=bf16(gv * np.asarray(wo, np.float64)),
        wcx=bf16(np.concatenate([wcx[:C], wcx[:C] + wcx[C:]], axis=0)),
        bcb=f32(bcrow).reshape(NCT, 128).T.copy(),
    )
    in_maps = []
    for core in range(8):
        b, h = core // 2, core % 2
        own = xf[b, h * N_OWN:(h + 1) * N_OWN]
        oth = xf[b, (1 - h) * N_OWN:(2 - h) * N_OWN]
        xp = np.concatenate([own, oth], axis=0)
        in_maps.append({"xt": bf16(np.ascontiguousarray(xp.T)), **common})
    return in_maps, B


def assemble(results, B):
    out = np.empty((B, N_FULL, C), np.float32)
    for core in range(8):
        b, h = core // 2, core % 2
        out[b, h * N_OWN:(h + 1) * N_OWN] = results[core]["y"].T
    return out.reshape(B, 64, 64, C)


def kernel(**inputs):
    in_maps, B = make_in_maps(**inputs)
    nc = _get_prog()
    res = run_bass_kernel_spmd(nc, in_maps, core_ids=list(range(8)))
    return assemble(res.results, B)
